# revision 45
# baseline (speedup 1.0000x reference)
"""Distributed RoPE-attention kernel for 8 TRN2 NeuronCores.

Problem: x[2,2048,1024]; q/k/v/o projections (1024x1024, bias-free),
16 heads x 64 dims, RoPE on q/k, softmax attention, o-projection.

Sharding:
  - Attention: head-parallel. Core i owns heads 2i, 2i+1 (rows
    128i:128(i+1) of Wq/Wk/Wv). Each core: QKV projections (bf16) ->
    RoPE -> attention for its 2 heads over both batches, transposed
    layout [head-dim x tokens].
  - o_proj: token-parallel. Core j owns 512 tokens, 128 from each
    1024-token group: tokens 1024c + [128j, 128(j+1)) for c in 0..3.
    Four AllToAll collectives (one per group, triggered as soon as its
    two query-blocks finish attention) redistribute the attention
    outputs from head-sharded to token-sharded layout (~0.25MB/core
    on the wire each, vs 7MB recv for an AllGather). The first three
    hide under remaining attention compute; only the fourth (~8-20us,
    mostly fixed cost) is exposed. A tiny dummy AllToAll at kernel
    start absorbs the CC-stream startup latency. Each core computes
    out[:, its tokens] with the full Wo; the host reassembles.

Softmax: scores ~ N(0,1) after the 1/sqrt(Dh) scale, so exp() without
max-subtraction is safe in f32. Denominators come for free from a
ones-column appended to V (M=65 matmul costs the same as M=64).

Schedule: ScalarE exp (128 x ~1.1us, serial) is the hard floor of the
attention phase, and the PE FIFO is strict program order. The whole
attention is emitted as one software-pipelined stream over (b,qb,kb)
steps: per step, scores+exp are emitted first and the PV matmuls of
the PREVIOUS step after, so the PE never blocks on the current exp
and ScalarE stays saturated. All remaining PE work (QKV units for
t2>=1, o_proj blocks, softmax normalizes) is chopped into small
closures pulled from a filler queue between steps, sized to the PE
idle slack per step. This also keeps the PE busy enough that the HAM
clock gate stays at full rate.

PSUM (8 banks x 2KB/partition, all in one pool, per-tag rings):
  sg scores   tag "big"  bufs=2  [128,1024]f32 -> 4 banks
  proj halves tag "proj" bufs=1  [128, 512]f32 -> 1 bank
  oe accum    tag "pv"   bufs=2  [128, 512]f32 -> 2 banks
  transients  tag "aux"  bufs=1  [128, 512]f32 -> 1 bank
Ring-reuse safety rule: a tag's ring is only reused when every op on
the tile `bufs` allocations back is already emitted. "aux" tiles
(rope-rot, normalize-broadcast, o_proj acc, v-transpose) are
allocated and fully consumed within a single closure; oe reuse is
safe because each qb's normalize is emitted (via the queue front)
before the next qb's first PV.
"""

import math
from collections import deque
import numpy as np
import ml_dtypes

import concourse.bacc as bacc
import concourse.mybir as mybir
import concourse.tile as tile
from concourse.bass import broadcast_tensor_aps as bass_broadcast
from concourse.bass_utils import run_bass_kernel_spmd

USE_DMA_BCAST = True

BF16 = mybir.dt.bfloat16
F32 = mybir.dt.float32
AF = mybir.ActivationFunctionType
ALU = mybir.AluOpType

N_CORES = 8
B, S, D = 2, 2048, 1024
H, DH = 16, 64
T = B * S               # 4096 tokens
HPC = H // N_CORES      # 2 heads per core
PC = HPC * DH           # 128 head-dims per core
TPC = T // N_CORES      # 512 tokens owned per core (for o_proj)

_CACHED = {}


def _rope_tables():
    inv_freq = 1.0 / (10000.0 ** (np.arange(0, DH, 2, dtype=np.float64) / DH))
    t = np.arange(S, dtype=np.float64)
    f = np.einsum("i,j->ij", t, inv_freq)          # [S, 32]
    freqs = np.concatenate([f, f], axis=-1)        # [S, 64]
    cos = np.cos(freqs).T.astype(np.float32)       # [64, S]
    sin = np.sin(freqs).T.astype(np.float32)
    cos2 = np.concatenate([cos, cos], axis=0)      # [128, S] (2 heads)
    sin2 = np.concatenate([sin, sin], axis=0)
    return cos2.astype(ml_dtypes.bfloat16), sin2.astype(ml_dtypes.bfloat16)


def _rotate_matrix_T():
    # R: per-64 block [[0,-I32],[I32,0]]  (rotate_half in column space)
    R = np.zeros((PC, PC), dtype=np.float32)
    for h in range(HPC):
        b0 = h * DH
        for i in range(32):
            R[b0 + i, b0 + 32 + i] = -1.0
            R[b0 + 32 + i, b0 + i] = 1.0
    return R.T.copy().astype(ml_dtypes.bfloat16)   # lhsT for PE


def build():
    nc = bacc.Bacc("TRN2", target_bir_lowering=False, debug=False,
                   num_devices=N_CORES)

    # weights arrive host-pre-laid in SBUF layout [128, c, m] flattened to
    # [128, c*m] so the DMA is contiguous 2KB+ lines per partition.
    xT = nc.declare_dram_parameter("xT", [D, T], BF16, isOutput=False)
    wq = nc.declare_dram_parameter("wq", [128, (D // 128) * PC], BF16, isOutput=False)
    wk = nc.declare_dram_parameter("wk", [128, (D // 128) * PC], BF16, isOutput=False)
    wv = nc.declare_dram_parameter("wv", [128, (D // 128) * PC], BF16, isOutput=False)
    wo = nc.declare_dram_parameter("wo", [128, (D // 128) * D], BF16, isOutput=False)
    out = nc.declare_dram_parameter("out", [D, TPC], F32, isOutput=True)

    cos_np, sin_np = _rope_tables()
    cos_d = nc.inline_tensor(cos_np, "cos_d")
    sin_d = nc.inline_tensor(sin_np, "sin_d")
    rt_d = nc.inline_tensor(_rotate_matrix_T(), "rt_d")
    id_d = nc.inline_tensor(np.eye(128, dtype=np.float32).astype(ml_dtypes.bfloat16), "id_d")
    ones_d = nc.inline_tensor(np.ones((1, DH), dtype=np.float32).astype(ml_dtypes.bfloat16), "ones_d")

    DC = D // 128           # 8 contraction chunks
    NQB = 4                 # 512-token query blocks per batch
    QB = S // NQB           # 512
    NKB = S // 128          # 16 key chunks per batch
    VW = HPC * (DH + 1)     # 130: packed v-normal layout (64 dims + ones) x 2

    with tile.TileContext(nc) as tc:
        with (
            tc.tile_pool(name="const", bufs=1) as constp,
            tc.tile_pool(name="resid", bufs=1) as resid,
            tc.tile_pool(name="rope", bufs=4) as ropep,
            tc.tile_pool(name="pp", bufs=6) as pp,
            tc.tile_pool(name="ogp", bufs=2) as ogp,
            tc.tile_pool(name="finp", bufs=4) as finp,
            tc.tile_pool(name="recp", bufs=4) as recp,
            tc.tile_pool(name="ps", bufs=1, space="PSUM") as psp,
            tc.tile_pool(name="dram", bufs=1, space="DRAM") as dram,
        ):
            # ---- load constants / inputs to SBUF (first MMs need wq + the
            # first half-t2 of x, so those DMAs go first) ----
            wq_sb = constp.tile([128, DC, PC], BF16, name="wq_sb")
            wk_sb = constp.tile([128, DC, PC], BF16, name="wk_sb")
            wv_sb = constp.tile([128, DC, PC], BF16, name="wv_sb")
            x_sb = resid.tile([128, DC, T], BF16)
            x_re = xT.ap().rearrange("(c p) m -> p c m", p=128)
            cos_sb = constp.tile([128, S], BF16)
            sin_sb = constp.tile([128, S], BF16)
            rt_sb = constp.tile([128, PC], BF16)
            id_sb = constp.tile([128, 128], BF16)
            ones_sb = constp.tile([1, DH], BF16)

            # The lead-in (first halves of k/v/q units over tokens 0:512)
            # needs wk/wv/wq + x[:, :, 0:512] + the rope tables; split those
            # transfers across the sync and gpsimd queues so the first
            # matmul unblocks during the engine-init preamble.
            nc.sync.dma_start(wk_sb[:], wk.ap().rearrange("p (c m) -> p c m", c=DC))
            nc.sync.dma_start(x_sb[:, 0:3, 0:512], x_re[:, 0:3, 0:512])
            nc.gpsimd.dma_start(x_sb[:, 3:6, 0:512], x_re[:, 3:6, 0:512])
            nc.scalar.dma_start(x_sb[:, 6:DC, 0:512], x_re[:, 6:DC, 0:512])
            nc.scalar.dma_start(cos_sb[:], cos_d[:])
            nc.scalar.dma_start(sin_sb[:], sin_d[:])
            nc.scalar.dma_start(rt_sb[:], rt_d[:])
            nc.sync.dma_start(wv_sb[:], wv.ap().rearrange("p (c m) -> p c m", c=DC))
            nc.gpsimd.dma_start(wq_sb[:], wq.ap().rearrange("p (c m) -> p c m", c=DC))
            nc.sync.dma_start(x_sb[:, 0:3, 512:1024], x_re[:, 0:3, 512:1024])
            nc.gpsimd.dma_start(x_sb[:, 3:6, 512:1024], x_re[:, 3:6, 512:1024])
            nc.scalar.dma_start(x_sb[:, 6:DC, 512:1024], x_re[:, 6:DC, 512:1024])
            nc.gpsimd.dma_start(id_sb[:], id_d[:])
            nc.gpsimd.dma_start(ones_sb[:], ones_d[:])
            for t2 in range(1, 4):
                nc.sync.dma_start(x_sb[:, :, t2 * 1024:(t2 + 1) * 1024],
                                  x_re[:, :, t2 * 1024:(t2 + 1) * 1024])

            wo_sb = constp.tile([128, DC, D], BF16)
            nc.sync.dma_start(wo_sb[:], wo.ap().rearrange("p (c m) -> p c m", c=DC))

            w_sb = {"q": wq_sb, "k": wk_sb, "v": wv_sb}

            qT_sb = resid.tile([128, T], BF16)
            kT_sb = resid.tile([128, T], BF16)
            vT_sb = resid.tile([128, T], BF16)
            # v in normal layout [token-part, (64 v-dims + ones-col) x 2 heads]
            vn_sb = resid.tile([128, T // 128, VW], BF16, name="vn_sb")
            nc.gpsimd.memset(vn_sb[:], 1.0)

            outT_sb = resid.tile([128, T], BF16)

            # preload the exp table-set during the DMA lead-in (~2.7us)
            warm = recp.tile([1, 2], F32, tag="dsb", name="warm")
            nc.gpsimd.memset(warm[:], 0.0)
            warm2 = recp.tile([1, 2], BF16, tag="recb", name="warm2")
            nc.scalar.activation(warm2[:], warm[:], AF.Exp)

            # ---- AllToAll buffers: chunks 0-2 cover 1024 tokens (2 qbs)
            # each, chunks 3 and 4 cover 512 tokens (qb (1,2) and (1,3)), so
            # the only exposed collective -- the last one -- is half-size.
            # Core j owns tokens base_c + [w_c * j, w_c * (j+1)) of chunk c.
            CHUNKS = [(0, 128), (1024, 128), (2048, 128), (3072, 64), (3584, 64)]
            CB = [0, 128, 256, 384, 448]       # out column base per chunk
            GQB_CH = {0: 0, 1: 0, 2: 1, 3: 1, 4: 2, 5: 2, 6: 3, 7: 4}
            a2a_in = [dram.tile([128 * N_CORES, w], BF16, name=f"a2a_in{c}")
                      for c, (_, w) in enumerate(CHUNKS)]
            a2a_out = [dram.tile([128 * N_CORES, w], BF16, name=f"a2a_out{c}")
                       for c, (_, w) in enumerate(CHUNKS)]
            # tiny dummy collective: absorbs CC-stream startup cost early
            wcc_in = dram.tile([N_CORES, 64], BF16, name="wcc_in")
            wcc_out = dram.tile([N_CORES, 64], BF16, name="wcc_out")
            nc.gpsimd.collective_compute(
                "AllToAll", ALU.bypass,
                replica_groups=[list(range(N_CORES))],
                ins=[wcc_in.opt()], outs=[wcc_out.opt()],
            )

            # ================= building blocks =================
            proj_ps = {}

            def emit_proj_mms(t2, nm, half, d0, alloc):
                ts = t2 * 1024 + half * 512
                if alloc:
                    proj_ps[(t2, nm, half)] = psp.tile(
                        [128, 512], F32, tag="proj", bufs=1,
                        name=f"ph_{t2}{nm}{half}")
                ph = proj_ps[(t2, nm, half)]
                for d in (d0, d0 + 1):
                    nc.tensor.matmul(
                        ph[:], w_sb[nm][:, d, :], x_sb[:, d, ts:ts + 512],
                        start=(d == 0), stop=(d == DC - 1),
                    )

            def emit_rope_half(t2, nm, half):
                ts = t2 * 1024 + half * 512
                ph = proj_ps.pop((t2, nm, half))
                dst = qT_sb if nm == "q" else kT_sb
                raw = ropep.tile([128, 512], BF16, tag="raw", name=f"raw{t2}{nm}{half}")
                nc.vector.tensor_copy(raw[:], ph[:])
                ss = ts % S
                tmp1 = ropep.tile([128, 512], BF16, tag="t1", name=f"t1_{t2}{nm}{half}")
                nc.vector.tensor_mul(tmp1[:], raw[:], cos_sb[:, ss:ss + 512])
                rot = psp.tile([128, 512], F32, tag="aux", bufs=1,
                               name=f"rot{t2}{nm}{half}")
                nc.tensor.matmul(rot[:], rt_sb[:], raw[:])
                tmp2 = ropep.tile([128, 512], BF16, tag="t2", name=f"t2_{t2}{nm}{half}")
                nc.vector.tensor_mul(tmp2[:], rot[:], sin_sb[:, ss:ss + 512])
                nc.vector.tensor_add(dst[:, ts:ts + 512], tmp1[:], tmp2[:])

            def emit_v_half(t2, half):
                ts = t2 * 1024 + half * 512
                ph = proj_ps.pop((t2, "v", half))
                nc.vector.tensor_copy(vT_sb[:, ts:ts + 512], ph[:])

            def emit_v_transpose(t2, half, cc0):
                for cc in (cc0, cc0 + 1):
                    c = t2 * 8 + half * 4 + cc
                    pt = psp.tile([128, 128], BF16, tag="aux", bufs=1,
                                  name=f"pt{c}")
                    nc.tensor.matmul(
                        pt[:], vT_sb[:, c * 128:(c + 1) * 128],
                        id_sb[:], is_transpose=True,
                    )
                    nc.vector.tensor_copy(
                        vn_sb[:, c, :].rearrange("p (h e) -> p h e", h=HPC)[:, :, 0:DH],
                        pt[:].rearrange("p (h e) -> p h e", h=HPC),
                    )

            def half_closures(t2, nm, half):
                """One 512-token half of a projection unit as small filler
                closures. PSUM tiles never outlive their half's closures."""
                cls = []
                for d0 in range(0, DC, 2):
                    cls.append(lambda t2=t2, nm=nm, half=half, d0=d0:
                               emit_proj_mms(t2, nm, half, d0, d0 == 0))
                if nm == "v":
                    cls.append(lambda t2=t2, half=half: emit_v_half(t2, half))
                    cls.append(lambda t2=t2, half=half: emit_v_transpose(t2, half, 0))
                    cls.append(lambda t2=t2, half=half: emit_v_transpose(t2, half, 2))
                else:
                    cls.append(lambda t2=t2, nm=nm, half=half:
                               emit_rope_half(t2, nm, half))
                return cls

            def unit_closures(t2, nm):
                return half_closures(t2, nm, 0) + half_closures(t2, nm, 1)

            def emit_half_now(t2, nm, half):
                for c in half_closures(t2, nm, half):
                    c()

            # -------- attention step pieces --------
            def emit_scores_exp(b, qb, kb):
                bs = b * S
                qs = bs + qb * QB
                ks = bs + kb * 128
                sg = psp.tile([128, 1024], F32, tag="big", bufs=2,
                              name=f"sg{b}{qb}{kb}")
                for h in range(HPC):
                    nc.tensor.matmul(
                        sg[:, h * QB:(h + 1) * QB],
                        kT_sb[h * DH:(h + 1) * DH, ks:ks + 128],
                        qT_sb[h * DH:(h + 1) * DH, qs:qs + QB],
                    )
                p = pp.tile([128, 1024], BF16, tag="p", name=f"p{b}{qb}{kb}")
                nc.scalar.activation(p[:], sg[:], AF.Exp,
                                     scale=1.0 / math.sqrt(DH))
                return p

            oe_cur = {}

            def emit_pv(b, qb, kb, p):
                if kb == 0:
                    oe_cur[(b, qb)] = [
                        psp.tile([128, QB], F32, tag="pv", bufs=2,
                                 name=f"oe{h}_{b}_{qb}")
                        for h in range(HPC)]
                oe = oe_cur[(b, qb)]
                kc = b * NKB + kb
                for h in range(HPC):
                    nc.tensor.matmul(
                        oe[h][0:DH + 1, :],
                        vn_sb[:, kc, h * (DH + 1):(h + 1) * (DH + 1)],
                        p[:, h * QB:(h + 1) * QB],
                        start=(kb == 0), stop=(kb == NKB - 1),
                    )

            def emit_normalize(b, qb, fast=False):
                qs = b * S + qb * QB
                oe = oe_cur.pop((b, qb))
                # Free oe as early as possible: its only readers are the
                # reciprocal of the denominator row and a bf16 copy of the
                # value rows, so the next qb's PV (which reuses the "pv"
                # PSUM ring) doesn't wait on the whole broadcast chain.
                att = None
                if not fast:
                    att = recp.tile([128, QB], BF16, tag="att", name=f"att{b}{qb}")
                rec = {}
                for h in range(HPC):
                    dsb = recp.tile([1, QB], F32, tag="dsb", name=f"dsb{b}{qb}{h}")
                    nc.vector.tensor_copy(dsb[:], oe[h][DH:DH + 1, :])
                    if not fast:
                        # free oe's PSUM ring early for the next qb's PV
                        nc.vector.tensor_copy(att[h * DH:(h + 1) * DH, :],
                                              oe[h][0:DH, :])
                    rec[h] = recp.tile([1, QB], F32, tag="rec", name=f"rec{b}{qb}{h}")
                    nc.vector.reciprocal_approx_fast(rec[h][:], dsb[:])
                bcs = recp.tile([128, QB], BF16, tag="bcs", name=f"bcs{b}{qb}")
                for h in range(HPC):
                    recb = recp.tile([1, QB], BF16, tag="recb", name=f"recb{b}{qb}{h}")
                    nc.vector.tensor_copy(recb[:], rec[h][:])
                    if fast or not USE_DMA_BCAST:
                        # PE ones-matmul broadcast: lower latency, used for
                        # the last qb where normalize is on the tail path.
                        bc = psp.tile([128, QB], F32, tag="aux", bufs=1,
                                      name=f"bc{b}{qb}{h}")
                        nc.tensor.matmul(bc[0:DH, :], ones_sb[:], recb[:])
                        nc.vector.tensor_copy(bcs[h * DH:(h + 1) * DH, :],
                                              bc[0:DH, :])
                    else:
                        # partition-broadcast via DRAM bounce (SBUF APs
                        # cannot have stride-0 partitions; DRAM reads can).
                        rd = dram.tile([1, QB], BF16, name=f"rd{b}{qb}{h}")
                        nc.gpsimd.dma_start(rd[:], recb[:])
                        dst2 = bcs[h * DH:(h + 1) * DH, :]
                        sbc, _ = bass_broadcast(rd[:], dst2)
                        nc.gpsimd.dma_start(dst2, sbc)
                for h in range(HPC):
                    src = oe[h][0:DH, :] if fast else att[h * DH:(h + 1) * DH, :]
                    nc.vector.tensor_mul(
                        outT_sb[h * DH:(h + 1) * DH, qs:qs + QB],
                        src, bcs[h * DH:(h + 1) * DH, :])

            # -------- A2A staging / o_proj --------
            def emit_a2a_stage(gqb):
                # One DMA per qb: scatter the 512 finished tokens of outT
                # into the owning cores' blocks of the chunk's A2A input.
                # Issue off the gpsimd queue (it carries the broadcast chains
                # and triggers): sync mid-phase, and the scalar queue -- idle
                # once the final exp is done -- for the last qb so the
                # exposed A2A triggers as early as possible.
                c = GQB_CH[gqb]
                base, w = CHUNKS[c]
                eng = nc.scalar if gqb == 7 else nc.sync
                nblk = QB // w
                blk0 = (gqb * QB - base) // w
                dst = a2a_in[c][blk0 * 128:(blk0 + nblk) * 128, :].rearrange(
                    "(blk p) m -> p blk m", p=128)
                src = outT_sb[:, gqb * QB:(gqb + 1) * QB].rearrange(
                    "p (blk m) -> p blk m", blk=nblk)
                eng.dma_start(dst, src)

            def emit_a2a(c):
                nc.gpsimd.collective_compute(
                    "AllToAll", ALU.bypass,
                    replica_groups=[list(range(N_CORES))],
                    ins=[a2a_in[c].opt()], outs=[a2a_out[c].opt()],
                )

            og_sb = {}

            def emit_og_load(c):
                w = CHUNKS[c][1]
                og = ogp.tile([128, DC, w], BF16, tag="og", name=f"og{c}")
                nc.sync.dma_start(
                    og[:], a2a_out[c][:].rearrange("(c p) m -> p c m", p=128))
                og_sb[c] = og

            def emit_oproj_blk(c, ob, tag="aux", bufs=1):
                w = CHUNKS[c][1]
                acc = psp.tile([128, w], F32, tag=tag, bufs=bufs,
                               name=f"acc{c}{ob}")
                for d in range(DC):
                    nc.tensor.matmul(acc[:], wo_sb[:, d, ob * 128:(ob + 1) * 128],
                                     og_sb[c][:, d, :],
                                     start=(d == 0), stop=(d == DC - 1))
                fin = finp.tile([128, w], F32, tag="fin", name=f"fin{c}{ob}")
                nc.vector.tensor_copy(fin[:], acc[:])
                nc.sync.dma_start(
                    out[ob * 128:(ob + 1) * 128, CB[c]:CB[c] + w], fin[:])

            # ================= schedule =================
            # Lead-in: just what attention(0,0) kb0-3 needs (first halves of
            # k/v/q for t2=0), emitted densely. The rest rides the queue in
            # dependency order; the budget-4 early pumping clears each piece
            # before the step that consumes it (verified by pop arithmetic).
            for nm in ("k", "v", "q"):
                emit_half_now(0, nm, 0)

            fq = deque()

            def pump(n):
                for _ in range(n):
                    if not fq:
                        return
                    fq.popleft()()

            for t2, nm, half in (
                (0, "k", 1), (1, "k", 0), (0, "v", 1), (1, "v", 0),
                (0, "q", 1), (1, "k", 1), (1, "v", 1), (1, "q", 0), (1, "q", 1),
            ):
                fq.extend(half_closures(t2, nm, half))
            for t2 in (2, 3):
                for nm in ("k", "v", "q"):
                    fq.extend(unit_closures(t2, nm))

            steps = [(b, qb, kb)
                     for b in range(B) for qb in range(NQB) for kb in range(NKB)]

            def qb_done_closure(pb, pqb):
                def qb_done():
                    emit_normalize(pb, pqb)
                    gqb = pb * NQB + pqb
                    emit_a2a_stage(gqb)
                    if gqb in (1, 3, 5, 6):
                        emit_a2a(GQB_CH[gqb])
                return qb_done

            def push_oproj(c):
                fq.append(lambda: emit_og_load(c))
                for ob in range(DC):
                    fq.append(lambda ob=ob: emit_oproj_blk(c, ob))

            pending = None   # (b, qb, kb, p-tile) awaiting PV emission
            for idx, (b, qb, kb) in enumerate(steps):
                budget = 3 if idx < 16 else (2 if idx < 48 else 1)
                p = emit_scores_exp(b, qb, kb)
                pump(budget)
                if pending is not None:
                    emit_pv(*pending)
                    pb, pqb, pkb = pending[0], pending[1], pending[2]
                    if pkb == NKB - 1:
                        # normalize + A2A staging/trigger ride the queue front
                        # so they run promptly after the qb finishes (and
                        # before the next qb's oe reuses the "pv" ring).
                        fq.appendleft(qb_done_closure(pb, pqb))
                pending = (b, qb, kb, p)

                # o_proj fillers, timed so each chunk's A2A (triggered after
                # qb 2c+1) has comfortably landed before its first consumer,
                # with margin for slow-collective runs.
                if (b, qb, kb) == (1, 1, 8):
                    push_oproj(0)
                if (b, qb, kb) == (1, 2, 12):
                    push_oproj(1)
                # chunk 2's o_proj is NOT pushed as a filler: its closures
                # would sit in the PE FIFO ahead of the final PVs and block
                # on A2A#2 completion, delaying the exposed last trigger.
                # It runs in the tail instead, overlapping A2A#3's flight.

            emit_pv(*pending)
            emit_normalize(1, 3, fast=True)
            emit_a2a_stage(7)
            emit_a2a(4)
            while fq:
                fq.popleft()()
            # chunks 2 and 3's o_proj fill the PE while the final (half-
            # size) A2A is in flight; then the last chunk. The oe "pv" ring
            # is free now, so alternate accumulators through it to avoid
            # serializing each block on the previous one's PSUM drain.
            for c in (2, 3):
                emit_og_load(c)
                for ob in range(DC):
                    emit_oproj_blk(c, ob, tag="pv", bufs=2)
            emit_og_load(4)
            for ob in range(DC):
                emit_oproj_blk(4, ob, tag="pv", bufs=2)

    nc.compile()
    return nc


def _get_nc():
    if "nc" not in _CACHED:
        _CACHED["nc"] = build()
    return _CACHED["nc"]


def _prep_w(Wm):
    # [D, M] (rows = contraction dim) -> SBUF layout [128, c, m] flattened
    # to [128, c*m], contiguous per partition.
    Dd, M = Wm.shape
    return np.ascontiguousarray(
        Wm.reshape(Dd // 128, 128, M).transpose(1, 0, 2).reshape(128, -1)
    ).astype(ml_dtypes.bfloat16)


def make_in_maps(x, Wq, Wk, Wv, Wo):
    xT = np.ascontiguousarray(
        np.asarray(x, dtype=np.float32).reshape(T, D).T).astype(ml_dtypes.bfloat16)
    woT = _prep_w(np.ascontiguousarray(np.asarray(Wo, dtype=np.float32).T))
    in_maps = []
    for c in range(N_CORES):
        r0, r1 = c * PC, (c + 1) * PC
        in_maps.append({
            "xT": xT,
            "wq": _prep_w(np.ascontiguousarray(np.asarray(Wq, np.float32)[r0:r1, :].T)),
            "wk": _prep_w(np.ascontiguousarray(np.asarray(Wk, np.float32)[r0:r1, :].T)),
            "wv": _prep_w(np.ascontiguousarray(np.asarray(Wv, np.float32)[r0:r1, :].T)),
            "wo": woT,
        })
    return in_maps


def assemble(outs):
    # outs[j]: [1024, 512] f32; chunk c's column block (base CB, width w)
    # holds tokens base_c + [w*j, w*(j+1)).
    chunks = [(0, 0, 128), (1024, 128, 128), (2048, 256, 128),
              (3072, 384, 64), (3584, 448, 64)]
    full = np.empty((T, D), dtype=np.float32)
    for j in range(N_CORES):
        o = outs[j]
        for base, cb, w in chunks:
            full[base + w * j:base + w * (j + 1), :] = o[:, cb:cb + w].T
    return np.ascontiguousarray(full.reshape(B, S, D))


def kernel(x, Wq, Wk, Wv, Wo):
    in_maps = make_in_maps(x, Wq, Wk, Wv, Wo)
    nc = _get_nc()
    res = run_bass_kernel_spmd(nc, in_maps, core_ids=list(range(N_CORES)))
    outs = [res.results[c]["out"] for c in range(N_CORES)]   # [1024, 512]
    return assemble(outs).astype(np.float32)


if __name__ == "__main__":
    rng = np.random.default_rng(0)
    ins = {
        "x": rng.standard_normal((B, S, D), dtype=np.float32),
        "Wq": rng.standard_normal((D, D), dtype=np.float32) / 32,
        "Wk": rng.standard_normal((D, D), dtype=np.float32) / 32,
        "Wv": rng.standard_normal((D, D), dtype=np.float32) / 32,
        "Wo": rng.standard_normal((D, D), dtype=np.float32) / 32,
    }
    o = kernel(**ins)
    print("kernel out:", o.shape, o.dtype, float(np.abs(o).mean()))


# revision 46
# speedup vs baseline: 1.0598x; 1.0598x over previous
"""Distributed RoPE-attention kernel for 8 TRN2 NeuronCores.

Problem: x[2,2048,1024]; q/k/v/o projections (1024x1024, bias-free),
16 heads x 64 dims, RoPE on q/k, softmax attention, o-projection.

Sharding:
  - Attention: head-parallel. Core i owns heads 2i, 2i+1 (rows
    128i:128(i+1) of Wq/Wk/Wv). Each core: QKV projections (bf16) ->
    RoPE -> attention for its 2 heads over both batches, transposed
    layout [head-dim x tokens].
  - o_proj: token-parallel. Core j owns 512 tokens, 128 from each
    1024-token group: tokens 1024c + [128j, 128(j+1)) for c in 0..3.
    Four AllToAll collectives (one per group, triggered as soon as its
    two query-blocks finish attention) redistribute the attention
    outputs from head-sharded to token-sharded layout (~0.25MB/core
    on the wire each, vs 7MB recv for an AllGather). The first three
    hide under remaining attention compute; only the fourth (~8-20us,
    mostly fixed cost) is exposed. A tiny dummy AllToAll at kernel
    start absorbs the CC-stream startup latency. Each core computes
    out[:, its tokens] with the full Wo; the host reassembles.

Softmax: scores ~ N(0,1) after the 1/sqrt(Dh) scale, so exp() without
max-subtraction is safe in f32. Denominators come for free from a
ones-column appended to V (M=65 matmul costs the same as M=64).

Schedule: ScalarE exp (128 x ~1.1us, serial) is the hard floor of the
attention phase, and the PE FIFO is strict program order. The whole
attention is emitted as one software-pipelined stream over (b,qb,kb)
steps: per step, scores+exp are emitted first and the PV matmuls of
the PREVIOUS step after, so the PE never blocks on the current exp
and ScalarE stays saturated. All remaining PE work (QKV units for
t2>=1, o_proj blocks, softmax normalizes) is chopped into small
closures pulled from a filler queue between steps, sized to the PE
idle slack per step. This also keeps the PE busy enough that the HAM
clock gate stays at full rate.

PSUM (8 banks x 2KB/partition, all in one pool, per-tag rings):
  sg scores   tag "big"  bufs=2  [128,1024]f32 -> 4 banks
  proj halves tag "proj" bufs=1  [128, 512]f32 -> 1 bank
  oe accum    tag "pv"   bufs=2  [128, 512]f32 -> 2 banks
  transients  tag "aux"  bufs=1  [128, 512]f32 -> 1 bank
Ring-reuse safety rule: a tag's ring is only reused when every op on
the tile `bufs` allocations back is already emitted. "aux" tiles
(rope-rot, normalize-broadcast, o_proj acc, v-transpose) are
allocated and fully consumed within a single closure; oe reuse is
safe because each qb's normalize is emitted (via the queue front)
before the next qb's first PV.
"""

import math
from collections import deque
import numpy as np
import ml_dtypes

import concourse.bacc as bacc
import concourse.mybir as mybir
import concourse.tile as tile
from concourse.bass import broadcast_tensor_aps as bass_broadcast
from concourse.bass_utils import run_bass_kernel_spmd

USE_DMA_BCAST = True

BF16 = mybir.dt.bfloat16
F32 = mybir.dt.float32
AF = mybir.ActivationFunctionType
ALU = mybir.AluOpType

N_CORES = 8
B, S, D = 2, 2048, 1024
H, DH = 16, 64
T = B * S               # 4096 tokens
HPC = H // N_CORES      # 2 heads per core
PC = HPC * DH           # 128 head-dims per core
TPC = T // N_CORES      # 512 tokens owned per core (for o_proj)

_CACHED = {}


def _rope_tables():
    inv_freq = 1.0 / (10000.0 ** (np.arange(0, DH, 2, dtype=np.float64) / DH))
    t = np.arange(S, dtype=np.float64)
    f = np.einsum("i,j->ij", t, inv_freq)          # [S, 32]
    freqs = np.concatenate([f, f], axis=-1)        # [S, 64]
    cos = np.cos(freqs).T.astype(np.float32)       # [64, S]
    sin = np.sin(freqs).T.astype(np.float32)
    cos2 = np.concatenate([cos, cos], axis=0)      # [128, S] (2 heads)
    sin2 = np.concatenate([sin, sin], axis=0)
    return cos2.astype(ml_dtypes.bfloat16), sin2.astype(ml_dtypes.bfloat16)


def _rotate_matrix_T():
    # R: per-64 block [[0,-I32],[I32,0]]  (rotate_half in column space)
    R = np.zeros((PC, PC), dtype=np.float32)
    for h in range(HPC):
        b0 = h * DH
        for i in range(32):
            R[b0 + i, b0 + 32 + i] = -1.0
            R[b0 + 32 + i, b0 + i] = 1.0
    return R.T.copy().astype(ml_dtypes.bfloat16)   # lhsT for PE


def build():
    nc = bacc.Bacc("TRN2", target_bir_lowering=False, debug=False,
                   num_devices=N_CORES)

    # weights arrive host-pre-laid in SBUF layout [128, c, m] flattened to
    # [128, c*m] so the DMA is contiguous 2KB+ lines per partition.
    xT = nc.declare_dram_parameter("xT", [D, T], BF16, isOutput=False)
    wq = nc.declare_dram_parameter("wq", [128, (D // 128) * PC], BF16, isOutput=False)
    wk = nc.declare_dram_parameter("wk", [128, (D // 128) * PC], BF16, isOutput=False)
    wv = nc.declare_dram_parameter("wv", [128, (D // 128) * PC], BF16, isOutput=False)
    wo = nc.declare_dram_parameter("wo", [128, (D // 128) * D], BF16, isOutput=False)
    out = nc.declare_dram_parameter("out", [D, TPC], F32, isOutput=True)

    cos_np, sin_np = _rope_tables()
    cos_d = nc.inline_tensor(cos_np, "cos_d")
    sin_d = nc.inline_tensor(sin_np, "sin_d")
    rt_d = nc.inline_tensor(_rotate_matrix_T(), "rt_d")
    id_d = nc.inline_tensor(np.eye(128, dtype=np.float32).astype(ml_dtypes.bfloat16), "id_d")
    ones_d = nc.inline_tensor(np.ones((1, DH), dtype=np.float32).astype(ml_dtypes.bfloat16), "ones_d")

    DC = D // 128           # 8 contraction chunks
    NQB = 4                 # 512-token query blocks per batch
    QB = S // NQB           # 512
    NKB = S // 128          # 16 key chunks per batch
    VW = HPC * (DH + 1)     # 130: packed v-normal layout (64 dims + ones) x 2

    with tile.TileContext(nc) as tc:
        with (
            tc.tile_pool(name="const", bufs=1) as constp,
            tc.tile_pool(name="resid", bufs=1) as resid,
            tc.tile_pool(name="rope", bufs=4) as ropep,
            tc.tile_pool(name="pp", bufs=6) as pp,
            tc.tile_pool(name="ogp", bufs=2) as ogp,
            tc.tile_pool(name="finp", bufs=4) as finp,
            tc.tile_pool(name="recp", bufs=4) as recp,
            tc.tile_pool(name="ps", bufs=1, space="PSUM") as psp,
            tc.tile_pool(name="dram", bufs=1, space="DRAM") as dram,
        ):
            # ---- load constants / inputs to SBUF (first MMs need wq + the
            # first half-t2 of x, so those DMAs go first) ----
            wq_sb = constp.tile([128, DC, PC], BF16, name="wq_sb")
            wk_sb = constp.tile([128, DC, PC], BF16, name="wk_sb")
            wv_sb = constp.tile([128, DC, PC], BF16, name="wv_sb")
            x_sb = resid.tile([128, DC, T], BF16)
            x_re = xT.ap().rearrange("(c p) m -> p c m", p=128)
            cos_sb = constp.tile([128, S], BF16)
            sin_sb = constp.tile([128, S], BF16)
            rt_sb = constp.tile([128, PC], BF16)
            id_sb = constp.tile([128, 128], BF16)
            ones_sb = constp.tile([1, DH], BF16)

            # The lead-in (first halves of k/v/q units over tokens 0:512)
            # needs wk/wv/wq + x[:, :, 0:512] + the rope tables; split those
            # transfers across the sync and gpsimd queues so the first
            # matmul unblocks during the engine-init preamble.
            nc.sync.dma_start(wk_sb[:], wk.ap().rearrange("p (c m) -> p c m", c=DC))
            nc.sync.dma_start(x_sb[:, 0:3, 0:512], x_re[:, 0:3, 0:512])
            nc.gpsimd.dma_start(x_sb[:, 3:6, 0:512], x_re[:, 3:6, 0:512])
            nc.scalar.dma_start(x_sb[:, 6:DC, 0:512], x_re[:, 6:DC, 0:512])
            nc.scalar.dma_start(cos_sb[:], cos_d[:])
            nc.scalar.dma_start(sin_sb[:], sin_d[:])
            nc.scalar.dma_start(rt_sb[:], rt_d[:])
            nc.sync.dma_start(wv_sb[:], wv.ap().rearrange("p (c m) -> p c m", c=DC))
            nc.gpsimd.dma_start(wq_sb[:], wq.ap().rearrange("p (c m) -> p c m", c=DC))
            nc.sync.dma_start(x_sb[:, 0:3, 512:1024], x_re[:, 0:3, 512:1024])
            nc.gpsimd.dma_start(x_sb[:, 3:6, 512:1024], x_re[:, 3:6, 512:1024])
            nc.scalar.dma_start(x_sb[:, 6:DC, 512:1024], x_re[:, 6:DC, 512:1024])
            nc.gpsimd.dma_start(id_sb[:], id_d[:])
            nc.gpsimd.dma_start(ones_sb[:], ones_d[:])
            for t2 in range(1, 4):
                nc.sync.dma_start(x_sb[:, :, t2 * 1024:(t2 + 1) * 1024],
                                  x_re[:, :, t2 * 1024:(t2 + 1) * 1024])

            wo_sb = constp.tile([128, DC, D], BF16)
            nc.sync.dma_start(wo_sb[:], wo.ap().rearrange("p (c m) -> p c m", c=DC))

            w_sb = {"q": wq_sb, "k": wk_sb, "v": wv_sb}

            qT_sb = resid.tile([128, T], BF16)
            kT_sb = resid.tile([128, T], BF16)
            vT_sb = resid.tile([128, T], BF16)
            # v in normal layout [token-part, (64 v-dims + ones-col) x 2 heads]
            vn_sb = resid.tile([128, T // 128, VW], BF16, name="vn_sb")
            nc.gpsimd.memset(vn_sb[:], 1.0)

            outT_sb = resid.tile([128, T], BF16)

            # preload the exp table-set during the DMA lead-in (~2.7us)
            warm = recp.tile([1, 2], F32, tag="dsb", name="warm")
            nc.gpsimd.memset(warm[:], 0.0)
            warm2 = recp.tile([1, 2], BF16, tag="recb", name="warm2")
            nc.scalar.activation(warm2[:], warm[:], AF.Exp)

            # ---- AllToAll buffers: chunks 0-2 cover 1024 tokens (2 qbs)
            # each, chunks 3 and 4 cover 512 tokens (qb (1,2) and (1,3)), so
            # the only exposed collective -- the last one -- is half-size.
            # Core j owns tokens base_c + [w_c * j, w_c * (j+1)) of chunk c.
            CHUNKS = [(0, 128), (1024, 128), (2048, 128), (3072, 64), (3584, 64)]
            CB = [0, 128, 256, 384, 448]       # out column base per chunk
            GQB_CH = {0: 0, 1: 0, 2: 1, 3: 1, 4: 2, 5: 2, 6: 3, 7: 4}
            a2a_in = [dram.tile([128 * N_CORES, w], BF16, name=f"a2a_in{c}")
                      for c, (_, w) in enumerate(CHUNKS)]
            a2a_out = [dram.tile([128 * N_CORES, w], BF16, name=f"a2a_out{c}")
                       for c, (_, w) in enumerate(CHUNKS)]
            # tiny dummy collective: absorbs CC-stream startup cost early
            wcc_in = dram.tile([N_CORES, 64], BF16, name="wcc_in")
            wcc_out = dram.tile([N_CORES, 64], BF16, name="wcc_out")
            nc.gpsimd.collective_compute(
                "AllToAll", ALU.bypass,
                replica_groups=[list(range(N_CORES))],
                ins=[wcc_in.opt()], outs=[wcc_out.opt()],
            )

            # ================= building blocks =================
            proj_ps = {}

            def emit_proj_mms(t2, nm, half, d0, alloc):
                ts = t2 * 1024 + half * 512
                if alloc:
                    proj_ps[(t2, nm, half)] = psp.tile(
                        [128, 512], F32, tag="proj", bufs=1,
                        name=f"ph_{t2}{nm}{half}")
                ph = proj_ps[(t2, nm, half)]
                for d in (d0, d0 + 1):
                    nc.tensor.matmul(
                        ph[:], w_sb[nm][:, d, :], x_sb[:, d, ts:ts + 512],
                        start=(d == 0), stop=(d == DC - 1),
                    )

            def emit_rope_half(t2, nm, half):
                ts = t2 * 1024 + half * 512
                ph = proj_ps.pop((t2, nm, half))
                dst = qT_sb if nm == "q" else kT_sb
                raw = ropep.tile([128, 512], BF16, tag="raw", name=f"raw{t2}{nm}{half}")
                nc.vector.tensor_copy(raw[:], ph[:])
                ss = ts % S
                tmp1 = ropep.tile([128, 512], BF16, tag="t1", name=f"t1_{t2}{nm}{half}")
                nc.vector.tensor_mul(tmp1[:], raw[:], cos_sb[:, ss:ss + 512])
                rot = psp.tile([128, 512], F32, tag="aux", bufs=1,
                               name=f"rot{t2}{nm}{half}")
                nc.tensor.matmul(rot[:], rt_sb[:], raw[:])
                tmp2 = ropep.tile([128, 512], BF16, tag="t2", name=f"t2_{t2}{nm}{half}")
                nc.vector.tensor_mul(tmp2[:], rot[:], sin_sb[:, ss:ss + 512])
                nc.vector.tensor_add(dst[:, ts:ts + 512], tmp1[:], tmp2[:])

            def emit_v_half(t2, half):
                ts = t2 * 1024 + half * 512
                ph = proj_ps.pop((t2, "v", half))
                nc.vector.tensor_copy(vT_sb[:, ts:ts + 512], ph[:])

            def emit_v_transpose(t2, half, cc0):
                for cc in (cc0, cc0 + 1):
                    c = t2 * 8 + half * 4 + cc
                    pt = psp.tile([128, 128], BF16, tag="aux", bufs=1,
                                  name=f"pt{c}")
                    nc.tensor.matmul(
                        pt[:], vT_sb[:, c * 128:(c + 1) * 128],
                        id_sb[:], is_transpose=True,
                    )
                    nc.vector.tensor_copy(
                        vn_sb[:, c, :].rearrange("p (h e) -> p h e", h=HPC)[:, :, 0:DH],
                        pt[:].rearrange("p (h e) -> p h e", h=HPC),
                    )

            def half_closures(t2, nm, half):
                """One 512-token half of a projection unit as small filler
                closures. PSUM tiles never outlive their half's closures."""
                cls = []
                for d0 in range(0, DC, 2):
                    cls.append(lambda t2=t2, nm=nm, half=half, d0=d0:
                               emit_proj_mms(t2, nm, half, d0, d0 == 0))
                if nm == "v":
                    cls.append(lambda t2=t2, half=half: emit_v_half(t2, half))
                    cls.append(lambda t2=t2, half=half: emit_v_transpose(t2, half, 0))
                    cls.append(lambda t2=t2, half=half: emit_v_transpose(t2, half, 2))
                else:
                    cls.append(lambda t2=t2, nm=nm, half=half:
                               emit_rope_half(t2, nm, half))
                return cls

            def unit_closures(t2, nm):
                return half_closures(t2, nm, 0) + half_closures(t2, nm, 1)

            def emit_half_now(t2, nm, half):
                for c in half_closures(t2, nm, half):
                    c()

            # -------- attention step pieces --------
            def emit_scores_exp(b, qb, kb):
                bs = b * S
                qs = bs + qb * QB
                ks = bs + kb * 128
                sg = psp.tile([128, 1024], F32, tag="big", bufs=2,
                              name=f"sg{b}{qb}{kb}")
                for h in range(HPC):
                    nc.tensor.matmul(
                        sg[:, h * QB:(h + 1) * QB],
                        kT_sb[h * DH:(h + 1) * DH, ks:ks + 128],
                        qT_sb[h * DH:(h + 1) * DH, qs:qs + QB],
                    )
                p = pp.tile([128, 1024], BF16, tag="p", name=f"p{b}{qb}{kb}")
                nc.scalar.activation(p[:], sg[:], AF.Exp,
                                     scale=1.0 / math.sqrt(DH))
                return p

            oe_cur = {}

            def emit_pv(b, qb, kb, p):
                if kb == 0:
                    oe_cur[(b, qb)] = [
                        psp.tile([128, QB], F32, tag="pv", bufs=2,
                                 name=f"oe{h}_{b}_{qb}")
                        for h in range(HPC)]
                oe = oe_cur[(b, qb)]
                kc = b * NKB + kb
                for h in range(HPC):
                    nc.tensor.matmul(
                        oe[h][0:DH + 1, :],
                        vn_sb[:, kc, h * (DH + 1):(h + 1) * (DH + 1)],
                        p[:, h * QB:(h + 1) * QB],
                        start=(kb == 0), stop=(kb == NKB - 1),
                    )

            def emit_normalize(b, qb, fast=False):
                qs = b * S + qb * QB
                oe = oe_cur.pop((b, qb))
                # Free oe as early as possible: its only readers are the
                # reciprocal of the denominator row and a bf16 copy of the
                # value rows, so the next qb's PV (which reuses the "pv"
                # PSUM ring) doesn't wait on the whole broadcast chain.
                att = None
                if not fast:
                    att = recp.tile([128, QB], BF16, tag="att", name=f"att{b}{qb}")
                rec = {}
                for h in range(HPC):
                    dsb = recp.tile([1, QB], F32, tag="dsb", name=f"dsb{b}{qb}{h}")
                    nc.vector.tensor_copy(dsb[:], oe[h][DH:DH + 1, :])
                    if not fast:
                        # free oe's PSUM ring early for the next qb's PV
                        nc.vector.tensor_copy(att[h * DH:(h + 1) * DH, :],
                                              oe[h][0:DH, :])
                    rec[h] = recp.tile([1, QB], F32, tag="rec", name=f"rec{b}{qb}{h}")
                    nc.vector.reciprocal_approx_fast(rec[h][:], dsb[:])
                bcs = recp.tile([128, QB], BF16, tag="bcs", name=f"bcs{b}{qb}")
                for h in range(HPC):
                    recb = recp.tile([1, QB], BF16, tag="recb", name=f"recb{b}{qb}{h}")
                    nc.vector.tensor_copy(recb[:], rec[h][:])
                    if fast or not USE_DMA_BCAST:
                        # PE ones-matmul broadcast: lower latency, used for
                        # the last qb where normalize is on the tail path.
                        bc = psp.tile([128, QB], F32, tag="aux", bufs=1,
                                      name=f"bc{b}{qb}{h}")
                        nc.tensor.matmul(bc[0:DH, :], ones_sb[:], recb[:])
                        nc.vector.tensor_copy(bcs[h * DH:(h + 1) * DH, :],
                                              bc[0:DH, :])
                    else:
                        # partition-broadcast via DRAM bounce (SBUF APs
                        # cannot have stride-0 partitions; DRAM reads can).
                        rd = dram.tile([1, QB], BF16, name=f"rd{b}{qb}{h}")
                        nc.gpsimd.dma_start(rd[:], recb[:])
                        dst2 = bcs[h * DH:(h + 1) * DH, :]
                        sbc, _ = bass_broadcast(rd[:], dst2)
                        nc.gpsimd.dma_start(dst2, sbc)
                for h in range(HPC):
                    src = oe[h][0:DH, :] if fast else att[h * DH:(h + 1) * DH, :]
                    nc.vector.tensor_mul(
                        outT_sb[h * DH:(h + 1) * DH, qs:qs + QB],
                        src, bcs[h * DH:(h + 1) * DH, :])

            # -------- A2A staging / o_proj --------
            def emit_a2a_stage(gqb):
                # One DMA per qb: scatter the 512 finished tokens of outT
                # into the owning cores' blocks of the chunk's A2A input.
                # Issue off the gpsimd queue (it carries the broadcast chains
                # and triggers): sync mid-phase, and the scalar queue -- idle
                # once the final exp is done -- for the last qb so the
                # exposed A2A triggers as early as possible.
                c = GQB_CH[gqb]
                base, w = CHUNKS[c]
                eng = nc.scalar if gqb == 7 else nc.sync
                nblk = QB // w
                blk0 = (gqb * QB - base) // w
                dst = a2a_in[c][blk0 * 128:(blk0 + nblk) * 128, :].rearrange(
                    "(blk p) m -> p blk m", p=128)
                src = outT_sb[:, gqb * QB:(gqb + 1) * QB].rearrange(
                    "p (blk m) -> p blk m", blk=nblk)
                eng.dma_start(dst, src)

            def emit_a2a(c):
                nc.gpsimd.collective_compute(
                    "AllToAll", ALU.bypass,
                    replica_groups=[list(range(N_CORES))],
                    ins=[a2a_in[c].opt()], outs=[a2a_out[c].opt()],
                )

            og_sb = {}

            def emit_og_load(c):
                w = CHUNKS[c][1]
                og = ogp.tile([128, DC, w], BF16, tag="og", name=f"og{c}")
                nc.sync.dma_start(
                    og[:], a2a_out[c][:].rearrange("(c p) m -> p c m", p=128))
                og_sb[c] = og

            def emit_oproj_blk(c, ob, tag="aux", bufs=1):
                w = CHUNKS[c][1]
                acc = psp.tile([128, w], F32, tag=tag, bufs=bufs,
                               name=f"acc{c}{ob}")
                for d in range(DC):
                    nc.tensor.matmul(acc[:], wo_sb[:, d, ob * 128:(ob + 1) * 128],
                                     og_sb[c][:, d, :],
                                     start=(d == 0), stop=(d == DC - 1))
                fin = finp.tile([128, w], F32, tag="fin", name=f"fin{c}{ob}")
                nc.vector.tensor_copy(fin[:], acc[:])
                nc.sync.dma_start(
                    out[ob * 128:(ob + 1) * 128, CB[c]:CB[c] + w], fin[:])

            # ================= schedule =================
            # Lead-in: just what attention(0,0) kb0-3 needs (first halves of
            # k/v/q for t2=0), emitted densely. The rest rides the queue in
            # dependency order; the budget-4 early pumping clears each piece
            # before the step that consumes it (verified by pop arithmetic).
            for nm in ("k", "v", "q"):
                emit_half_now(0, nm, 0)

            fq = deque()

            def pump(n):
                for _ in range(n):
                    if not fq:
                        return
                    fq.popleft()()

            for t2, nm, half in (
                (0, "k", 1), (1, "k", 0), (0, "v", 1), (1, "v", 0),
                (0, "q", 1), (1, "k", 1), (1, "v", 1), (1, "q", 0), (1, "q", 1),
            ):
                fq.extend(half_closures(t2, nm, half))
            for t2 in (2, 3):
                for nm in ("k", "v", "q"):
                    fq.extend(unit_closures(t2, nm))

            steps = [(b, qb, kb)
                     for b in range(B) for qb in range(NQB) for kb in range(NKB)]

            def qb_done_closure(pb, pqb):
                def qb_done():
                    emit_normalize(pb, pqb)
                    gqb = pb * NQB + pqb
                    emit_a2a_stage(gqb)
                    if gqb in (1, 3, 5, 6):
                        emit_a2a(GQB_CH[gqb])
                return qb_done

            def push_oproj(c):
                fq.append(lambda: emit_og_load(c))
                for ob in range(DC):
                    fq.append(lambda ob=ob: emit_oproj_blk(c, ob))

            pending = None   # (b, qb, kb, p-tile) awaiting PV emission
            for idx, (b, qb, kb) in enumerate(steps):
                budget = 3 if idx < 16 else (2 if idx < 48 else 1)
                p = emit_scores_exp(b, qb, kb)
                pump(budget)
                if pending is not None:
                    emit_pv(*pending)
                    pb, pqb, pkb = pending[0], pending[1], pending[2]
                    if pkb == NKB - 1:
                        # normalize + A2A staging/trigger ride the queue front
                        # so they run promptly after the qb finishes (and
                        # before the next qb's oe reuses the "pv" ring).
                        fq.appendleft(qb_done_closure(pb, pqb))
                pending = (b, qb, kb, p)

                # o_proj fillers, timed so each chunk's A2A (triggered after
                # qb 2c+1) has comfortably landed before its first consumer,
                # with margin for slow-collective runs.
                if (b, qb, kb) == (1, 1, 8):
                    push_oproj(0)
                if (b, qb, kb) == (1, 2, 12):
                    push_oproj(1)
                # chunk 2's o_proj is NOT pushed as a filler: its closures
                # would sit in the PE FIFO ahead of the final PVs and block
                # on A2A#2 completion, delaying the exposed last trigger.
                # It runs in the tail instead, overlapping A2A#3's flight.

            emit_pv(*pending)
            emit_normalize(1, 3, fast=True)
            emit_a2a_stage(7)
            emit_a2a(4)
            while fq:
                fq.popleft()()
            # chunks 2 and 3's o_proj fill the PE while the final (half-
            # size) A2A is in flight; then the last chunk. The oe "pv" ring
            # is free now, so alternate accumulators through it to avoid
            # serializing each block on the previous one's PSUM drain.
            for c in (2, 3):
                emit_og_load(c)
                for ob in range(DC):
                    emit_oproj_blk(c, ob, tag="pv", bufs=2)
            # last chunk: accumulate all 8 output blocks into one fin tile
            # and write them with a single DMA (8 separate issues would eat
            # ~4us of serial queue time at the very end).
            emit_og_load(4)
            w4 = CHUNKS[4][1]
            fin4 = finp.tile([128, DC, w4], F32, tag="fin4", name="fin4")
            for ob in range(DC):
                acc = psp.tile([128, w4], F32, tag="pv", bufs=2,
                               name=f"acc4{ob}")
                for d in range(DC):
                    nc.tensor.matmul(acc[:], wo_sb[:, d, ob * 128:(ob + 1) * 128],
                                     og_sb[4][:, d, :],
                                     start=(d == 0), stop=(d == DC - 1))
                nc.vector.tensor_copy(fin4[:, ob, :], acc[:])
            nc.sync.dma_start(
                out[:, CB[4]:CB[4] + w4].rearrange("(c p) m -> p c m", p=128),
                fin4[:])

    nc.compile()
    return nc


def _get_nc():
    if "nc" not in _CACHED:
        _CACHED["nc"] = build()
    return _CACHED["nc"]


def _prep_w(Wm):
    # [D, M] (rows = contraction dim) -> SBUF layout [128, c, m] flattened
    # to [128, c*m], contiguous per partition.
    Dd, M = Wm.shape
    return np.ascontiguousarray(
        Wm.reshape(Dd // 128, 128, M).transpose(1, 0, 2).reshape(128, -1)
    ).astype(ml_dtypes.bfloat16)


def make_in_maps(x, Wq, Wk, Wv, Wo):
    xT = np.ascontiguousarray(
        np.asarray(x, dtype=np.float32).reshape(T, D).T).astype(ml_dtypes.bfloat16)
    woT = _prep_w(np.ascontiguousarray(np.asarray(Wo, dtype=np.float32).T))
    in_maps = []
    for c in range(N_CORES):
        r0, r1 = c * PC, (c + 1) * PC
        in_maps.append({
            "xT": xT,
            "wq": _prep_w(np.ascontiguousarray(np.asarray(Wq, np.float32)[r0:r1, :].T)),
            "wk": _prep_w(np.ascontiguousarray(np.asarray(Wk, np.float32)[r0:r1, :].T)),
            "wv": _prep_w(np.ascontiguousarray(np.asarray(Wv, np.float32)[r0:r1, :].T)),
            "wo": woT,
        })
    return in_maps


def assemble(outs):
    # outs[j]: [1024, 512] f32; chunk c's column block (base CB, width w)
    # holds tokens base_c + [w*j, w*(j+1)).
    chunks = [(0, 0, 128), (1024, 128, 128), (2048, 256, 128),
              (3072, 384, 64), (3584, 448, 64)]
    full = np.empty((T, D), dtype=np.float32)
    for j in range(N_CORES):
        o = outs[j]
        for base, cb, w in chunks:
            full[base + w * j:base + w * (j + 1), :] = o[:, cb:cb + w].T
    return np.ascontiguousarray(full.reshape(B, S, D))


def kernel(x, Wq, Wk, Wv, Wo):
    in_maps = make_in_maps(x, Wq, Wk, Wv, Wo)
    nc = _get_nc()
    res = run_bass_kernel_spmd(nc, in_maps, core_ids=list(range(N_CORES)))
    outs = [res.results[c]["out"] for c in range(N_CORES)]   # [1024, 512]
    return assemble(outs).astype(np.float32)


if __name__ == "__main__":
    rng = np.random.default_rng(0)
    ins = {
        "x": rng.standard_normal((B, S, D), dtype=np.float32),
        "Wq": rng.standard_normal((D, D), dtype=np.float32) / 32,
        "Wk": rng.standard_normal((D, D), dtype=np.float32) / 32,
        "Wv": rng.standard_normal((D, D), dtype=np.float32) / 32,
        "Wo": rng.standard_normal((D, D), dtype=np.float32) / 32,
    }
    o = kernel(**ins)
    print("kernel out:", o.shape, o.dtype, float(np.abs(o).mean()))
